# revision 13
# baseline (speedup 1.0000x reference)
"""Trainium2 Bass kernel for grouped single-query MHA (ragged segment attention).

Data-parallel over the item axis N across 8 NeuronCores; per-class partial
sums on each core + AllReduce over classes; epilogue (softmax divide +
output projection) on device.

Self-contained: hardcodes all shapes. kernel(**inputs) -> (v_final, c_final).
"""

import sys

for _p in ("/opt/trn_rl_repo",):
    if _p not in sys.path:
        sys.path.insert(0, _p)

import numpy as np

import concourse.bacc as bacc
import concourse.bass as bass
import concourse.mybir as mybir
import concourse.tile as tile
from concourse.masks import make_identity

F32 = mybir.dt.float32
F16 = mybir.dt.float16
BF16 = mybir.dt.bfloat16
I16 = mybir.dt.int16

N_ITEMS = 524288
N_CLASSES = 1024
EMB = 64
HEADS = 4
HD = EMB // HEADS  # 16
N_CORES = 8
SCALE = 1.0 / np.sqrt(HD)  # 0.25

AF = mybir.ActivationFunctionType
ALU = mybir.AluOpType


def build_kernel(ns: int, n_cores: int = N_CORES):
    """Build the per-core Bass graph. ns = items per core per stream."""
    assert ns % 512 == 0
    n_sup = ns // 512  # supertiles of 512 tokens (4 tiles of 128)

    nc = bacc.Bacc(
        "TRN2",
        target_bir_lowering=False,
        debug=False,
        num_devices=n_cores,
    )

    # ---------------- I/O ----------------
    ins = {}
    for s in ("v", "c"):
        ins[f"f{s}"] = nc.dram_tensor(f"f{s}", [ns, EMB], F32, kind="ExternalInput")
        ins[f"gi_{s}"] = nc.dram_tensor(
            f"gi_{s}", [128, ns // 16], I16, kind="ExternalInput"
        )
        ins[f"cls_{s}"] = nc.dram_tensor(
            f"cls_{s}", [128, ns // 128], F32, kind="ExternalInput"
        )
        ins[f"sem_{s}"] = nc.dram_tensor(
            f"sem_{s}", [N_CLASSES, EMB], F32, kind="ExternalInput"
        )
        ins[f"inw_{s}"] = nc.dram_tensor(
            f"inw_{s}", [3 * EMB, EMB], F32, kind="ExternalInput"
        )
        ins[f"inb_{s}"] = nc.dram_tensor(
            f"inb_{s}", [3 * EMB], F32, kind="ExternalInput"
        )
        ins[f"outw_{s}"] = nc.dram_tensor(
            f"outw_{s}", [EMB, EMB], F32, kind="ExternalInput"
        )
        ins[f"outb_{s}"] = nc.dram_tensor(
            f"outb_{s}", [EMB], F32, kind="ExternalInput"
        )
    out_t = nc.dram_tensor("out", [2, N_CLASSES, EMB], F32, kind="ExternalOutput")

    # internal DRAM
    r_dram = {
        s: nc.dram_tensor(f"r_dram_{s}", [N_CLASSES, HEADS * EMB], BF16)
        for s in ("v", "c")
    }
    cc_in = nc.dram_tensor("cc_in", [2 * 68, N_CLASSES], F32)
    cc_out = nc.dram_tensor(
        "cc_out",
        [2 * 68, N_CLASSES],
        F32,
        addr_space="Shared" if n_cores > 4 else "Local",
    )

    with tile.TileContext(nc) as tc:
        with (
            tc.tile_pool(name="const", bufs=1) as constp,
            tc.tile_pool(name="persist", bufs=1) as persist,
        ):
            # ---------- constants ----------
            ident = constp.tile([128, 128], BF16, name="ident")
            make_identity(nc, ident)
            iota_row = constp.tile([128, N_CLASSES], I16, name="iota_row")
            nc.gpsimd.iota(iota_row, pattern=[[1, N_CLASSES]], base=0, channel_multiplier=0)
            identf32 = constp.tile([128, 128], F32, name="identf32")
            make_identity(nc, identf32)

            # persistent per-stream sbuf state
            gi_sb = {}
            cls_sb = {}
            wvt_ext = {}
            owt_ext = {}
            for s in ("v", "c"):
                gi = persist.tile([128, ns // 16], I16, name=f"gi_sb_{s}")
                nc.sync.dma_start(out=gi, in_=ins[f"gi_{s}"].ap())
                gi_sb[s] = gi
                cl = persist.tile([128, ns // 128], F32, name=f"cls_sb_{s}")
                nc.sync.dma_start(out=cl, in_=ins[f"cls_{s}"].ap())
                cls_sb[s] = cl
                wvt_ext[s] = persist.tile([65, EMB], BF16, name=f"wvt_ext_{s}")
                owt_ext[s] = persist.tile([65, EMB], BF16, name=f"owt_ext_{s}")

            # ---------- weight prep (tiny) ----------
            with (
                tc.tile_pool(name="prep_sb", bufs=2) as prep_sb,
                tc.tile_pool(name="prep_ps", bufs=2, space="PSUM") as prep_ps,
            ):
                idf = prep_sb.tile([128, 128], F32, name="idf")
                make_identity(nc, idf)
                for s in ("v", "c"):
                    # load in_w [192,64] as two tiles
                    inw0 = prep_sb.tile([128, EMB], F32, name="inw0")  # Wq rows 0:64, Wk 64:128
                    nc.sync.dma_start(out=inw0, in_=ins[f"inw_{s}"].ap()[0:128, :])

                    # transpose Wq -> WqT [64(i),64(e)]
                    wqt_ps = prep_ps.tile([64, 128], F32, name="wqt_ps", tag="pp", space="PSUM")
                    nc.tensor.transpose(
                        out=wqt_ps[:, 0:64], in_=inw0[0:64, :], identity=idf[0:64, 0:64]
                    )
                    wqt = prep_sb.tile([64, 64], F32, name="wqt")
                    nc.scalar.copy(out=wqt, in_=wqt_ps[:, 0:64])

                    # semT [64, 1024]
                    semt = prep_sb.tile([64, N_CLASSES], F32, name="semt")
                    for j in range(8):
                        st_ps = prep_ps.tile([64, 128], F32, name="st_ps", tag="pp", space="PSUM")
                        sem_tile = prep_sb.tile([128, EMB], F32, name="sem_tile")
                        nc.sync.dma_start(
                            out=sem_tile,
                            in_=ins[f"sem_{s}"].ap()[128 * j : 128 * (j + 1), :],
                        )
                        nc.tensor.transpose(out=st_ps, in_=sem_tile, identity=idf)
                        nc.scalar.copy(out=semt[:, 128 * j : 128 * (j + 1)], in_=st_ps)

                    # qT [64, 1024] = WqT.T @ semT + bq
                    qt_ps = prep_ps.tile([64, N_CLASSES], F32, name="qt_ps", tag="pp", space="PSUM")
                    for h in range(2):
                        nc.tensor.matmul(
                            out=qt_ps[:, 512 * h : 512 * (h + 1)],
                            lhsT=wqt,
                            rhs=semt[:, 512 * h : 512 * (h + 1)],
                            start=True,
                            stop=True,
                        )
                    bq = prep_sb.tile([64, 1], F32, name="bq")
                    nc.sync.dma_start(
                        out=bq, in_=ins[f"inb_{s}"].ap()[0:64].unsqueeze(1)
                    )
                    qt = prep_sb.tile([64, N_CLASSES], F32, name="qt")
                    nc.vector.tensor_scalar(
                        out=qt, in0=qt_ps, scalar1=bq, scalar2=None, op0=ALU.add
                    )

                    # block-diag BDK [64, 256]: BDK[16h+d, 64h+i] = Wk[16h+d, i]
                    bdk = prep_sb.tile([64, HEADS * EMB], F32, name="bdk")
                    nc.vector.memset(bdk, 0.0)
                    for h in range(HEADS):
                        # partition-shifting copy via sbuf->sbuf DMA
                        nc.sync.dma_start(
                            out=bdk[16 * h : 16 * (h + 1), 64 * h : 64 * (h + 1)],
                            in_=inw0[64 + 16 * h : 80 + 16 * h, :],
                        )

                    # R chunks: [128c, 256] = qT_chunk.T @ BDK, scaled, -> bf16 -> dram
                    for j in range(8):
                        r_ps = prep_ps.tile([128, HEADS * EMB], F32, name="r_ps", tag="pp", space="PSUM")
                        nc.tensor.matmul(
                            out=r_ps,
                            lhsT=qt[:, 128 * j : 128 * (j + 1)],
                            rhs=bdk,
                            start=True,
                            stop=True,
                        )
                        r_sb = prep_sb.tile([128, HEADS * EMB], BF16, name="r_sb")
                        nc.scalar.activation(out=r_sb, in_=r_ps, func=AF.Copy, scale=SCALE)
                        nc.sync.dma_start(
                            out=r_dram[s].ap()[128 * j : 128 * (j + 1), :], in_=r_sb
                        )

                    # WvT_ext [65, 64]: rows 0:64 = Wv^T, row 64 = bv
                    inw2 = prep_sb.tile([64, EMB], F32, name="inw2")  # Wv rows 128:192
                    nc.sync.dma_start(out=inw2, in_=ins[f"inw_{s}"].ap()[128:192, :])
                    wvt_ps = prep_ps.tile([64, 128], F32, name="wvt_ps", tag="pp", space="PSUM")
                    nc.tensor.transpose(
                        out=wvt_ps[:, 0:64], in_=inw2, identity=idf[0:64, 0:64]
                    )
                    nc.scalar.copy(out=wvt_ext[s][0:64, :], in_=wvt_ps[:, 0:64])
                    bv = prep_sb.tile([1, EMB], F32, name="bv")
                    nc.sync.dma_start(
                        out=bv, in_=ins[f"inb_{s}"].ap()[128:192].unsqueeze(0)
                    )
                    nc.vector.tensor_copy(out=wvt_ext[s][64:65, :], in_=bv)

                    # OWT_ext [65, 64]: rows 0:64 = out_w^T, row 64 = out_b
                    ow = prep_sb.tile([64, EMB], F32, name="ow")
                    nc.sync.dma_start(out=ow, in_=ins[f"outw_{s}"].ap())
                    owt_ps = prep_ps.tile([64, 128], F32, name="owt_ps", tag="pp", space="PSUM")
                    nc.tensor.transpose(out=owt_ps[:, 0:64], in_=ow, identity=idf[0:64, 0:64])
                    nc.scalar.copy(out=owt_ext[s][0:64, :], in_=owt_ps[:, 0:64])
                    ob = prep_sb.tile([1, EMB], F32, name="ob")
                    nc.sync.dma_start(out=ob, in_=ins[f"outb_{s}"].ap().unsqueeze(0))
                    nc.vector.tensor_copy(out=owt_ext[s][64:65, :], in_=ob)

            # ---------- main loop ----------
            with tc.tile_pool(name="acc_ps", bufs=1, space="PSUM") as acc_ps:
                ot = {
                    s: acc_ps.tile([68, N_CLASSES], F32, name=f"ot_{s}")
                    for s in ("v", "c")
                }
                with (
                    tc.tile_pool(name="lf32", bufs=3) as lf32,
                    tc.tile_pool(name="lgr", bufs=3) as lgr,
                    tc.tile_pool(name="lfx", bufs=3) as lfx,
                    tc.tile_pool(name="lft", bufs=2) as lft,
                    tc.tile_pool(name="lpr", bufs=2) as lpr,
                    tc.tile_pool(name="lst", bufs=2) as lst,
                    tc.tile_pool(name="lw", bufs=2) as lw,
                    tc.tile_pool(name="loh", bufs=3) as loh,
                    tc.tile_pool(name="ftps", bufs=2, space="PSUM") as ftps,
                    tc.tile_pool(name="vps_p", bufs=2, space="PSUM") as vps_p,
                ):
                    for s in ("v", "c"):
                        f_ap = ins[f"f{s}"].ap()
                        for st in range(n_sup):
                            t0 = 512 * st
                            # load feats [128, 4, 64] f32 : [p, j, :] = feats[t0+128j+p]
                            f32t = lf32.tile([128, 4, EMB], F32, name="f32t")
                            nc.sync.dma_start(
                                out=f32t,
                                in_=f_ap[t0 : t0 + 512, :].rearrange(
                                    "(p j) i -> p j i", p=128
                                ),
                            )
                            # gather R rows -> [128, 4, 256] bf16
                            gr = lgr.tile([128, 4, HEADS * EMB], BF16, name="gr")
                            nc.gpsimd.dma_gather(
                                out_ap=gr,
                                in_ap=r_dram[s].ap(),
                                idxs_ap=gi_sb[s][:, 32 * st : 32 * st + 32],
                                num_idxs=512,
                                num_idxs_reg=512,
                                elem_size=HEADS * EMB,
                            )
                            # cast to bf16 + ones column
                            fx = lfx.tile([128, 4, 65], BF16, name="fx")
                            nc.scalar.copy(out=fx[:, :, 0:64], in_=f32t)
                            nc.vector.memset(fx[:, :, 64:65], 1.0)

                            # transpose -> ftp [65, 4, 128] bf16 psum
                            ftp = ftps.tile([65, 4, 128], BF16, name="ftp", space="PSUM")
                            for j in range(4):
                                nc.tensor.transpose(
                                    out=ftp[:, j, :], in_=fx[:, j, :], identity=ident
                                )
                            ft_sb = lft.tile([65, 512], BF16, name="ft_sb")
                            nc.scalar.copy(out=ft_sb, in_=ftp.rearrange("p j i -> p (j i)"))

                            # scores: prod = gr * feats (bcast over heads), reduce
                            pr = lpr.tile([128, 4, HEADS * EMB], BF16, name="pr")
                            nc.vector.tensor_tensor(
                                out=pr,
                                in0=gr,
                                in1=fx[:, :, 0:64]
                                .unsqueeze(2)
                                .broadcast_to([128, 4, HEADS, EMB]),
                                op=ALU.mult,
                            )
                            s_tok = lst.tile([128, 4, HEADS], F16, name="s_tok")
                            nc.vector.tensor_reduce(
                                out=s_tok,
                                in_=pr.rearrange("p j (h i) -> p (j h) i", h=HEADS),
                                axis=mybir.AxisListType.X,
                                op=ALU.add,
                            )
                            # w_ext[:, :, 64:68] = exp(s)
                            w_ext = lw.tile([128, 4, 68], BF16, name="w_ext")
                            nc.scalar.activation(
                                out=w_ext[:, :, 64:68], in_=s_tok, func=AF.Exp
                            )

                            # v projection per tile: vps [128, 4, 64] f32
                            vps = vps_p.tile([128, 4, EMB], F32, name="vps", space="PSUM")
                            for j in range(4):
                                nc.tensor.matmul(
                                    out=vps[:, j, :],
                                    lhsT=ft_sb[:, 128 * j : 128 * (j + 1)],
                                    rhs=wvt_ext[s],
                                    start=True,
                                    stop=True,
                                )
                            # w = v * p (bcast over head slots of 16)
                            nc.vector.tensor_tensor(
                                out=w_ext[:, :, 0:64].rearrange(
                                    "p j (h d) -> p j h d", h=HEADS
                                ),
                                in0=vps.rearrange("p j (h d) -> p j h d", h=HEADS),
                                in1=w_ext[:, :, 64:68]
                                .unsqueeze(3)
                                .broadcast_to([128, 4, HEADS, HD]),
                                op=ALU.mult,
                            )

                            # scatter: onehot + accumulate MMs
                            for j in range(4):
                                oh = loh.tile([128, N_CLASSES], BF16, name="oh")
                                oh_eng = nc.scalar if (4 * st + j) % 4 == 3 else nc.vector
                                oh_eng.tensor_scalar(
                                    out=oh,
                                    in0=iota_row,
                                    scalar1=cls_sb[s][:, 4 * st + j : 4 * st + j + 1],
                                    scalar2=None,
                                    op0=ALU.is_equal,
                                )
                                first = st == 0 and j == 0
                                last = st == n_sup - 1 and j == 3
                                for h in range(2):
                                    nc.tensor.matmul(
                                        out=ot[s][:, 512 * h : 512 * (h + 1)],
                                        lhsT=w_ext[:, j, :],
                                        rhs=oh[:, 512 * h : 512 * (h + 1)],
                                        start=first,
                                        stop=last,
                                        skip_group_check=True,
                                    )

                # copy partials to sbuf + dram
                with tc.tile_pool(name="par_sb", bufs=2) as par_sb:
                    for si, s in enumerate(("v", "c")):
                        ot_sb = par_sb.tile([68, N_CLASSES], F32, name="ot_sb")
                        nc.scalar.copy(out=ot_sb, in_=ot[s])
                        nc.sync.dma_start(
                            out=cc_in.ap()[68 * si : 68 * (si + 1), :], in_=ot_sb
                        )

            # ---------- all-reduce + epilogue ----------
            nc.gpsimd.collective_compute(
                "AllReduce",
                ALU.add,
                ins=[cc_in.ap()],
                outs=[cc_out.ap()],
                replica_groups=[list(range(n_cores))],
            )

            with (
                tc.tile_pool(name="ep_sb", bufs=3) as ep_sb,
                tc.tile_pool(name="ep_ps", bufs=3, space="PSUM") as ep_ps,
            ):
                for si, s in enumerate(("v", "c")):
                    otr = ep_sb.tile([68, N_CLASSES], F32, name="otr")
                    nc.sync.dma_start(
                        out=otr, in_=cc_out.ap()[68 * si : 68 * (si + 1), :]
                    )
                    for j in range(8):
                        # transpose chunk -> [128c, 68] psum
                        och_ps = ep_ps.tile([128, 68], F32, name="och_ps", tag="ep", space="PSUM")
                        och_in = otr[:, 128 * j : 128 * (j + 1)]
                        nc.tensor.transpose(out=och_ps, in_=och_in, identity=identf32[0:68, 0:68])
                        # recip z
                        rz = ep_sb.tile([128, HEADS], F32, name="rz")
                        nc.vector.reciprocal(out=rz, in_=och_ps[:, 64:68])
                        # o = pv * rz (bcast 16) -> bf16, with ones col
                        obf = ep_sb.tile([128, 65], BF16, name="obf")
                        nc.vector.tensor_tensor(
                            out=obf[:, 0:64].rearrange("p (h d) -> p h d", h=HEADS),
                            in0=och_ps[:, 0:64].rearrange(
                                "p (h d) -> p h d", h=HEADS
                            ),
                            in1=rz.unsqueeze(2).broadcast_to([128, HEADS, HD]),
                            op=ALU.mult,
                        )
                        nc.vector.memset(obf[:, 64:65], 1.0)
                        # transpose -> [65, 128]
                        obt_ps = ep_ps.tile([65, 128], BF16, name="obt_ps", tag="ep", space="PSUM")
                        nc.tensor.transpose(out=obt_ps, in_=obf, identity=ident)
                        obt = ep_sb.tile([65, 128], BF16, name="obt")
                        nc.scalar.copy(out=obt, in_=obt_ps)
                        # final = obt.T @ owt_ext
                        fin_ps = ep_ps.tile([128, EMB], F32, name="fin_ps", tag="ep", space="PSUM")
                        nc.tensor.matmul(
                            out=fin_ps, lhsT=obt, rhs=owt_ext[s], start=True, stop=True
                        )
                        fin = ep_sb.tile([128, EMB], F32, name="fin")
                        nc.vector.tensor_copy(out=fin, in_=fin_ps)
                        nc.sync.dma_start(
                            out=out_t.ap()[si, 128 * j : 128 * (j + 1), :], in_=fin
                        )

    nc.compile()
    return nc


def _prep_core_inputs(inputs, core, ns):
    """Slice + marshal inputs for one core."""
    lo = core * ns
    hi = lo + ns
    m = {}
    for s, feats_name, cls_name, pre in (
        ("v", "v_s", "v_class", "var"),
        ("c", "c_s", "c_class", "con"),
    ):
        cls = np.asarray(inputs[cls_name][lo:hi]).astype(np.int16)
        m[f"f{s}"] = np.ascontiguousarray(np.asarray(inputs[feats_name][lo:hi], dtype=np.float32))
        wrapped = cls.reshape(ns // 16, 16).T  # [16, ns/16]
        m[f"gi_{s}"] = np.ascontiguousarray(np.tile(wrapped, (8, 1)))  # [128, ns/16]
        m[f"cls_{s}"] = np.ascontiguousarray(
            cls.reshape(ns // 128, 128).T.astype(np.float32)
        )  # [128, ns/128]
        m[f"sem_{s}"] = np.asarray(inputs[f"{'v' if s == 'v' else 'c'}_sem"], dtype=np.float32)
        m[f"inw_{s}"] = np.asarray(inputs[f"in_w_{pre}"], dtype=np.float32)
        m[f"inb_{s}"] = np.asarray(inputs[f"in_b_{pre}"], dtype=np.float32)
        m[f"outw_{s}"] = np.asarray(inputs[f"out_w_{pre}"], dtype=np.float32)
        m[f"outb_{s}"] = np.asarray(inputs[f"out_b_{pre}"], dtype=np.float32)
    return m


_CACHED = {}


def run(inputs, trace=False, **kw):
    from concourse.bass_utils import run_bass_kernel_spmd

    ns = N_ITEMS // N_CORES
    if "nc" not in _CACHED:
        _CACHED["nc"] = build_kernel(ns)
    nc = _CACHED["nc"]
    in_maps = [_prep_core_inputs(inputs, core, ns) for core in range(N_CORES)]
    return run_bass_kernel_spmd(
        nc, in_maps, core_ids=list(range(N_CORES)), trace=trace, **kw
    )


def kernel(**inputs):
    res = run(inputs)
    out = res.results[0]["out"]
    return (
        np.asarray(out[0], dtype=np.float32),
        np.asarray(out[1], dtype=np.float32),
    )


if __name__ == "__main__":
    pass


# revision 20
# speedup vs baseline: 31.4270x; 31.4270x over previous
"""Trainium2 Bass kernel for grouped single-query MHA (ragged segment attention).

Data-parallel over the item axis N across 8 NeuronCores; per-class partial
sums on each core + AllReduce over classes; epilogue (softmax divide +
output projection) on device.

Self-contained: hardcodes all shapes. kernel(**inputs) -> (v_final, c_final).
"""

import sys

for _p in ("/opt/trn_rl_repo",):
    if _p not in sys.path:
        sys.path.insert(0, _p)

import numpy as np

import concourse.bacc as bacc
import concourse.bass as bass
import concourse.mybir as mybir
import concourse.tile as tile
from concourse.masks import make_identity

F32 = mybir.dt.float32
F16 = mybir.dt.float16
BF16 = mybir.dt.bfloat16
I16 = mybir.dt.int16

N_ITEMS = 524288
N_CLASSES = 1024
EMB = 64
HEADS = 4
HD = EMB // HEADS  # 16
N_CORES = 8
SCALE = 1.0 / np.sqrt(HD)  # 0.25

SCORE_FM = True  # feature-major scoring (PE reduce) vs token-major DVE reduce

AF = mybir.ActivationFunctionType
ALU = mybir.AluOpType


def build_kernel(ns: int, n_cores: int = N_CORES, repeat: int = 1):
    """Build the per-core Bass graph. ns = items per core per stream.

    repeat>1 re-runs the main loop (for timing calibration); since both the
    weighted-v sums and z scale by `repeat`, the normalized output is
    unchanged."""
    assert ns % 512 == 0
    n_sup = ns // 512  # supertiles of 512 tokens (4 tiles of 128)

    nc = bacc.Bacc(
        "TRN2",
        target_bir_lowering=False,
        debug=False,
        num_devices=n_cores,
    )

    # ---------------- I/O ----------------
    ins = {}
    for s in ("v", "c"):
        ins[f"f{s}"] = nc.dram_tensor(f"f{s}", [ns, EMB], F32, kind="ExternalInput")
        ins[f"gi_{s}"] = nc.dram_tensor(
            f"gi_{s}", [128, ns // 16], I16, kind="ExternalInput"
        )
        ins[f"cls_{s}"] = nc.dram_tensor(
            f"cls_{s}", [128, ns // 128], F32, kind="ExternalInput"
        )
        ins[f"sem_{s}"] = nc.dram_tensor(
            f"sem_{s}", [N_CLASSES, EMB], F32, kind="ExternalInput"
        )
        ins[f"inw_{s}"] = nc.dram_tensor(
            f"inw_{s}", [3 * EMB, EMB], F32, kind="ExternalInput"
        )
        ins[f"inb_{s}"] = nc.dram_tensor(
            f"inb_{s}", [3 * EMB], F32, kind="ExternalInput"
        )
        ins[f"outw_{s}"] = nc.dram_tensor(
            f"outw_{s}", [EMB, EMB], F32, kind="ExternalInput"
        )
        ins[f"outb_{s}"] = nc.dram_tensor(
            f"outb_{s}", [EMB], F32, kind="ExternalInput"
        )
    out_t = nc.dram_tensor("out", [2, N_CLASSES, EMB], F32, kind="ExternalOutput")

    # internal DRAM
    r_dram = {
        s: nc.dram_tensor(f"r_dram_{s}", [N_CLASSES, HEADS * EMB], BF16)
        for s in ("v", "c")
    }
    q_dram = {
        s: nc.dram_tensor(f"q_dram_{s}", [N_CLASSES, 128], BF16) for s in ("v", "c")
    }
    cc_in = nc.dram_tensor("cc_in", [2 * 68, N_CLASSES], F32)
    cc_out = nc.dram_tensor(
        "cc_out",
        [2 * 68, N_CLASSES],
        F32,
        addr_space="Shared" if n_cores > 4 else "Local",
    )

    with tile.TileContext(nc) as tc:
        with (
            tc.tile_pool(name="const", bufs=1) as constp,
            tc.tile_pool(name="persist", bufs=1) as persist,
        ):
            # ---------- constants ----------
            ident = constp.tile([128, 128], BF16, name="ident")
            make_identity(nc, ident)
            iota_row = constp.tile([128, N_CLASSES], I16, name="iota_row")
            nc.gpsimd.iota(iota_row, pattern=[[1, N_CLASSES]], base=0, channel_multiplier=0)
            identf32 = constp.tile([128, 128], F32, name="identf32")
            make_identity(nc, identf32)

            # persistent per-stream sbuf state
            gi_sb = {}
            cls_sb = {}
            wvt_ext = {}
            owt_ext = {}
            for s in ("v", "c"):
                gi = persist.tile([128, ns // 16], I16, name=f"gi_sb_{s}")
                nc.sync.dma_start(out=gi, in_=ins[f"gi_{s}"].ap())
                gi_sb[s] = gi
                cl = persist.tile([128, ns // 128], F32, name=f"cls_sb_{s}")
                nc.sync.dma_start(out=cl, in_=ins[f"cls_{s}"].ap())
                cls_sb[s] = cl
                wvt_ext[s] = persist.tile([65, EMB], BF16, name=f"wvt_ext_{s}")
                owt_ext[s] = persist.tile([65, EMB], BF16, name=f"owt_ext_{s}")
            wkt_bf = {
                s: persist.tile([64, EMB], BF16, name=f"wkt_bf_{s}") for s in ("v", "c")
            }
            # bd_ones[i, h] = 1 iff i // 16 == h (block-diag head-sum matrix)
            bd_ones = constp.tile([64, HEADS], BF16, name="bd_ones")
            nc.gpsimd.memset(bd_ones, 1.0)
            nc.gpsimd.affine_select(
                out=bd_ones, in_=bd_ones, compare_op=ALU.is_ge, fill=0.0,
                base=0, channel_multiplier=1, pattern=[[-16, HEADS]],
            )
            nc.gpsimd.affine_select(
                out=bd_ones, in_=bd_ones, compare_op=ALU.is_le, fill=0.0,
                base=-15, channel_multiplier=1, pattern=[[-16, HEADS]],
            )

            # ---------- weight prep (tiny) ----------
            with (
                tc.tile_pool(name="prep_sb", bufs=2) as prep_sb,
                tc.tile_pool(name="prep_ps", bufs=2, space="PSUM") as prep_ps,
            ):
                idf = prep_sb.tile([128, 128], F32, name="idf")
                make_identity(nc, idf)
                for s in ("v", "c"):
                    # load in_w [192,64] as two tiles
                    inw0 = prep_sb.tile([128, EMB], F32, name="inw0")  # Wq rows 0:64, Wk 64:128
                    nc.sync.dma_start(out=inw0, in_=ins[f"inw_{s}"].ap()[0:128, :])

                    # transpose Wq -> WqT [64(i),64(e)]
                    wqt_ps = prep_ps.tile([64, 128], F32, name="wqt_ps", tag="pp", space="PSUM")
                    nc.tensor.transpose(
                        out=wqt_ps[:, 0:64], in_=inw0[0:64, :], identity=idf[0:64, 0:64]
                    )
                    wqt = prep_sb.tile([64, 64], F32, name="wqt")
                    nc.scalar.copy(out=wqt, in_=wqt_ps[:, 0:64])

                    # semT [64, 1024]
                    semt = prep_sb.tile([64, N_CLASSES], F32, name="semt")
                    for j in range(8):
                        st_ps = prep_ps.tile([64, 128], F32, name="st_ps", tag="pp", space="PSUM")
                        sem_tile = prep_sb.tile([128, EMB], F32, name="sem_tile")
                        nc.sync.dma_start(
                            out=sem_tile,
                            in_=ins[f"sem_{s}"].ap()[128 * j : 128 * (j + 1), :],
                        )
                        nc.tensor.transpose(out=st_ps, in_=sem_tile, identity=idf)
                        nc.scalar.copy(out=semt[:, 128 * j : 128 * (j + 1)], in_=st_ps)

                    # qT [64, 1024] = WqT.T @ semT + bq
                    qt_ps = prep_ps.tile([64, N_CLASSES], F32, name="qt_ps", tag="pp", space="PSUM")
                    for h in range(2):
                        nc.tensor.matmul(
                            out=qt_ps[:, 512 * h : 512 * (h + 1)],
                            lhsT=wqt,
                            rhs=semt[:, 512 * h : 512 * (h + 1)],
                            start=True,
                            stop=True,
                        )
                    bq = prep_sb.tile([64, 1], F32, name="bq")
                    nc.sync.dma_start(
                        out=bq, in_=ins[f"inb_{s}"].ap()[0:64].unsqueeze(1)
                    )
                    qt = prep_sb.tile([64, N_CLASSES], F32, name="qt")
                    nc.vector.tensor_scalar(
                        out=qt, in0=qt_ps, scalar1=bq, scalar2=None, op0=ALU.add
                    )

                    # block-diag BDK [64, 256]: BDK[16h+d, 64h+i] = Wk[16h+d, i]
                    bdk = prep_sb.tile([64, HEADS * EMB], F32, name="bdk")
                    nc.vector.memset(bdk, 0.0)
                    for h in range(HEADS):
                        # partition-shifting copy via sbuf->sbuf DMA
                        nc.sync.dma_start(
                            out=bdk[16 * h : 16 * (h + 1), 64 * h : 64 * (h + 1)],
                            in_=inw0[64 + 16 * h : 80 + 16 * h, :],
                        )

                    if SCORE_FM:
                        # WkT bf16 for feature-major kT projection
                        wkt_ps = prep_ps.tile([64, 128], F32, name="wkt_ps", tag="pp", space="PSUM")
                        nc.tensor.transpose(
                            out=wkt_ps[:, 0:64],
                            in_=inw0[64:128, :],
                            identity=idf[64:128, 64:128],
                        )
                        nc.scalar.copy(out=wkt_bf[s], in_=wkt_ps[:, 0:64])
                        # q table rows [c, 0:64] = SCALE * q[c, :], cols 64:128 zero
                        qz = prep_sb.tile([128, 128], BF16, name="qz")
                        nc.vector.memset(qz, 0.0)
                        for j in range(8):
                            qch_ps = prep_ps.tile([128, 64], F32, name="qch_ps", tag="pp", space="PSUM")
                            nc.tensor.transpose(
                                out=qch_ps,
                                in_=qt[:, 128 * j : 128 * (j + 1)],
                                identity=idf[0:64, 0:64],
                            )
                            nc.scalar.activation(
                                out=qz[:, 0:64], in_=qch_ps, func=AF.Copy, scale=SCALE
                            )
                            nc.sync.dma_start(
                                out=q_dram[s].ap()[128 * j : 128 * (j + 1), :], in_=qz
                            )
                    else:
                      # R chunks: [128c, 256] = qT_chunk.T @ BDK, scaled, -> bf16 -> dram
                      for j in range(8):
                        r_ps = prep_ps.tile([128, HEADS * EMB], F32, name="r_ps", tag="pp", space="PSUM")
                        nc.tensor.matmul(
                            out=r_ps,
                            lhsT=qt[:, 128 * j : 128 * (j + 1)],
                            rhs=bdk,
                            start=True,
                            stop=True,
                        )
                        r_sb = prep_sb.tile([128, HEADS * EMB], BF16, name="r_sb")
                        nc.scalar.activation(out=r_sb, in_=r_ps, func=AF.Copy, scale=SCALE)
                        nc.sync.dma_start(
                            out=r_dram[s].ap()[128 * j : 128 * (j + 1), :], in_=r_sb
                        )

                    # WvT_ext [65, 64]: rows 0:64 = Wv^T, row 64 = bv
                    inw2 = prep_sb.tile([64, EMB], F32, name="inw2")  # Wv rows 128:192
                    nc.sync.dma_start(out=inw2, in_=ins[f"inw_{s}"].ap()[128:192, :])
                    wvt_ps = prep_ps.tile([64, 128], F32, name="wvt_ps", tag="pp", space="PSUM")
                    nc.tensor.transpose(
                        out=wvt_ps[:, 0:64], in_=inw2, identity=idf[0:64, 0:64]
                    )
                    nc.scalar.copy(out=wvt_ext[s][0:64, :], in_=wvt_ps[:, 0:64])
                    bv = prep_sb.tile([1, EMB], F32, name="bv")
                    nc.sync.dma_start(
                        out=bv, in_=ins[f"inb_{s}"].ap()[128:192].unsqueeze(0)
                    )
                    nc.vector.tensor_copy(out=wvt_ext[s][64:65, :], in_=bv)

                    # OWT_ext [65, 64]: rows 0:64 = out_w^T, row 64 = out_b
                    ow = prep_sb.tile([64, EMB], F32, name="ow")
                    nc.sync.dma_start(out=ow, in_=ins[f"outw_{s}"].ap())
                    owt_ps = prep_ps.tile([64, 128], F32, name="owt_ps", tag="pp", space="PSUM")
                    nc.tensor.transpose(out=owt_ps[:, 0:64], in_=ow, identity=idf[0:64, 0:64])
                    nc.scalar.copy(out=owt_ext[s][0:64, :], in_=owt_ps[:, 0:64])
                    ob = prep_sb.tile([1, EMB], F32, name="ob")
                    nc.sync.dma_start(out=ob, in_=ins[f"outb_{s}"].ap().unsqueeze(0))
                    nc.vector.tensor_copy(out=owt_ext[s][64:65, :], in_=ob)

            # ---------- main loop ----------
            with tc.tile_pool(name="acc_ps", bufs=1, space="PSUM") as acc_ps:
                ot = {
                    s: acc_ps.tile([68, N_CLASSES], F32, name=f"ot_{s}")
                    for s in ("v", "c")
                }
                with (
                    tc.tile_pool(name="lf32", bufs=3) as lf32,
                    tc.tile_pool(name="lgr", bufs=3) as lgr,
                    tc.tile_pool(name="lfx", bufs=3) as lfx,
                    tc.tile_pool(name="lft", bufs=2) as lft,
                    tc.tile_pool(name="lpr", bufs=2) as lpr,
                    tc.tile_pool(name="lst", bufs=2) as lst,
                    tc.tile_pool(name="lw", bufs=2) as lw,
                    tc.tile_pool(name="loh", bufs=3) as loh,
                    tc.tile_pool(name="ftps", bufs=1 if SCORE_FM else 2, space="PSUM") as ftps,
                    tc.tile_pool(name="vps_p", bufs=1 if SCORE_FM else 2, space="PSUM") as vps_p,
                    tc.tile_pool(name="ktps", bufs=1, space="PSUM") as ktps,
                    tc.tile_pool(name="spps", bufs=1, space="PSUM") as spps,
                ):
                    for rep in range(repeat):
                      for s in ("v", "c"):
                        f_ap = ins[f"f{s}"].ap()
                        for st in range(n_sup):
                            t0 = 512 * st
                            # load feats [128, 4, 64] f32 : [p, j, :] = feats[t0+128j+p]
                            f32t = lf32.tile([128, 4, EMB], F32, name="f32t")
                            nc.sync.dma_start(
                                out=f32t,
                                in_=f_ap[t0 : t0 + 512, :].rearrange(
                                    "(p j) i -> p j i", p=128
                                ),
                            )
                            if SCORE_FM:
                                # transpose-gather q rows -> gt [128, 1, 512] bf16
                                gt = lgr.tile([128, 1, 512], BF16, name="gt")
                                nc.gpsimd.dma_gather(
                                    out_ap=gt,
                                    in_ap=q_dram[s].ap(),
                                    idxs_ap=gi_sb[s][:, 32 * st : 32 * st + 32],
                                    num_idxs=512,
                                    num_idxs_reg=512,
                                    elem_size=128,
                                    transpose=True,
                                )
                            else:
                                # gather R rows -> [128, 4, 256] bf16
                                gr = lgr.tile([128, 4, HEADS * EMB], BF16, name="gr")
                                nc.gpsimd.dma_gather(
                                    out_ap=gr,
                                    in_ap=r_dram[s].ap(),
                                    idxs_ap=gi_sb[s][:, 32 * st : 32 * st + 32],
                                    num_idxs=512,
                                    num_idxs_reg=512,
                                    elem_size=HEADS * EMB,
                                )
                            # cast to bf16 + ones column
                            fx = lfx.tile([128, 4, 65], BF16, name="fx")
                            nc.scalar.copy(out=fx[:, :, 0:64], in_=f32t)
                            nc.vector.memset(fx[:, :, 64:65], 1.0)

                            # transpose -> ftp [65, 4, 128] bf16 psum
                            ftp = ftps.tile([65, 4, 128], BF16, name="ftp", space="PSUM")
                            for j in range(4):
                                nc.tensor.transpose(
                                    out=ftp[:, j, :], in_=fx[:, j, :], identity=ident
                                )
                            ft_sb = lft.tile([65, 512], BF16, name="ft_sb")
                            nc.scalar.copy(out=ft_sb, in_=ftp.rearrange("p j i -> p (j i)"))

                            w_ext = lw.tile([128, 4, 68], BF16, name="w_ext")
                            if SCORE_FM:
                                # kT [64, 512] = WkT.T @ featsT
                                kt_ps = ktps.tile([64, 512], F32, name="kt_ps", space="PSUM")
                                nc.tensor.matmul(
                                    out=kt_ps,
                                    lhsT=wkt_bf[s],
                                    rhs=ft_sb[0:64, :],
                                    start=True,
                                    stop=True,
                                )
                                # prodT = kT * q[cls]^T (feature-major)
                                prT = lpr.tile([64, 512], BF16, name="prT")
                                nc.vector.tensor_tensor(
                                    out=prT,
                                    in0=kt_ps,
                                    in1=gt[0:64, 0, :],
                                    op=ALU.mult,
                                )
                                # head-sum via block-diag ones -> sT [4, 512]
                                st_ps = spps.tile([4, 512], F32, name="st_ps", tag="sp", space="PSUM")
                                nc.tensor.matmul(
                                    out=st_ps,
                                    lhsT=bd_ones,
                                    rhs=prT,
                                    start=True,
                                    stop=True,
                                )
                                pT = lst.tile([4, 512], BF16, name="pT")
                                nc.scalar.activation(out=pT, in_=st_ps, func=AF.Exp)
                                # transpose p to token-major [128, 4(j), 4(h)]
                                p_ps = spps.tile([128, 4, HEADS], BF16, name="p_ps", tag="sp", space="PSUM")
                                for j in range(4):
                                    nc.tensor.transpose(
                                        out=p_ps[:, j, :],
                                        in_=pT[:, 128 * j : 128 * (j + 1)],
                                        identity=ident[0:4, 0:4],
                                    )
                                nc.vector.tensor_copy(
                                    out=w_ext[:, :, 64:68], in_=p_ps
                                )
                            else:
                                # scores: prod = gr * feats (bcast over heads), reduce
                                pr = lpr.tile([128, 4, HEADS * EMB], BF16, name="pr")
                                nc.vector.tensor_tensor(
                                    out=pr,
                                    in0=gr,
                                    in1=fx[:, :, 0:64]
                                    .unsqueeze(2)
                                    .broadcast_to([128, 4, HEADS, EMB]),
                                    op=ALU.mult,
                                )
                                s_tok = lst.tile([128, 4, HEADS], F16, name="s_tok")
                                with nc.allow_low_precision(
                                    reason="fp16 score out; |s|<6, fp32 internal accum"
                                ):
                                    nc.vector.tensor_reduce(
                                        out=s_tok,
                                        in_=pr.rearrange("p j (h i) -> p (j h) i", h=HEADS),
                                        axis=mybir.AxisListType.X,
                                        op=ALU.add,
                                    )
                                # w_ext[:, :, 64:68] = exp(s)
                                nc.scalar.activation(
                                    out=w_ext[:, :, 64:68], in_=s_tok, func=AF.Exp
                                )

                            # v projection per tile: vps [128, 4, 64] f32
                            vps = vps_p.tile([128, 4, EMB], F32, name="vps", space="PSUM")
                            for j in range(4):
                                nc.tensor.matmul(
                                    out=vps[:, j, :],
                                    lhsT=ft_sb[:, 128 * j : 128 * (j + 1)],
                                    rhs=wvt_ext[s],
                                    start=True,
                                    stop=True,
                                )
                            # w = v * p (bcast over head slots of 16)
                            nc.vector.tensor_tensor(
                                out=w_ext[:, :, 0:64].rearrange(
                                    "p j (h d) -> p j h d", h=HEADS
                                ),
                                in0=vps.rearrange("p j (h d) -> p j h d", h=HEADS),
                                in1=w_ext[:, :, 64:68]
                                .unsqueeze(3)
                                .broadcast_to([128, 4, HEADS, HD]),
                                op=ALU.mult,
                            )

                            # scatter: onehot + accumulate MMs
                            for j in range(4):
                                oh = loh.tile([128, N_CLASSES], BF16, name="oh")
                                nc.vector.tensor_scalar(
                                    out=oh,
                                    in0=iota_row,
                                    scalar1=cls_sb[s][:, 4 * st + j : 4 * st + j + 1],
                                    scalar2=None,
                                    op0=ALU.is_equal,
                                )
                                first = rep == 0 and st == 0 and j == 0
                                last = (
                                    rep == repeat - 1 and st == n_sup - 1 and j == 3
                                )
                                for h in range(2):
                                    nc.tensor.matmul(
                                        out=ot[s][:, 512 * h : 512 * (h + 1)],
                                        lhsT=w_ext[:, j, :],
                                        rhs=oh[:, 512 * h : 512 * (h + 1)],
                                        start=first,
                                        stop=last,
                                        skip_group_check=True,
                                    )

                # copy partials to sbuf + dram
                with tc.tile_pool(name="par_sb", bufs=2) as par_sb:
                    for si, s in enumerate(("v", "c")):
                        ot_sb = par_sb.tile([68, N_CLASSES], F32, name="ot_sb")
                        nc.scalar.copy(out=ot_sb, in_=ot[s])
                        nc.sync.dma_start(
                            out=cc_in.ap()[68 * si : 68 * (si + 1), :], in_=ot_sb
                        )

            # ---------- all-reduce + epilogue ----------
            nc.gpsimd.collective_compute(
                "AllReduce",
                ALU.add,
                ins=[cc_in.ap()],
                outs=[cc_out.ap()],
                replica_groups=[list(range(n_cores))],
            )

            with (
                tc.tile_pool(name="ep_sb", bufs=3) as ep_sb,
                tc.tile_pool(name="ep_ps", bufs=3, space="PSUM") as ep_ps,
            ):
                for si, s in enumerate(("v", "c")):
                    otr = ep_sb.tile([68, N_CLASSES], F32, name="otr")
                    nc.sync.dma_start(
                        out=otr, in_=cc_out.ap()[68 * si : 68 * (si + 1), :]
                    )
                    for j in range(8):
                        # transpose chunk -> [128c, 68] psum
                        och_ps = ep_ps.tile([128, 68], F32, name="och_ps", tag="ep", space="PSUM")
                        och_in = otr[:, 128 * j : 128 * (j + 1)]
                        nc.tensor.transpose(out=och_ps, in_=och_in, identity=identf32[0:68, 0:68])
                        # recip z
                        rz = ep_sb.tile([128, HEADS], F32, name="rz")
                        nc.vector.reciprocal(out=rz, in_=och_ps[:, 64:68])
                        # o = pv * rz (bcast 16) -> bf16, with ones col
                        obf = ep_sb.tile([128, 65], BF16, name="obf")
                        nc.vector.tensor_tensor(
                            out=obf[:, 0:64].rearrange("p (h d) -> p h d", h=HEADS),
                            in0=och_ps[:, 0:64].rearrange(
                                "p (h d) -> p h d", h=HEADS
                            ),
                            in1=rz.unsqueeze(2).broadcast_to([128, HEADS, HD]),
                            op=ALU.mult,
                        )
                        nc.vector.memset(obf[:, 64:65], 1.0)
                        # transpose -> [65, 128]
                        obt_ps = ep_ps.tile([65, 128], BF16, name="obt_ps", tag="ep", space="PSUM")
                        nc.tensor.transpose(out=obt_ps, in_=obf, identity=ident)
                        obt = ep_sb.tile([65, 128], BF16, name="obt")
                        nc.scalar.copy(out=obt, in_=obt_ps)
                        # final = obt.T @ owt_ext
                        fin_ps = ep_ps.tile([128, EMB], F32, name="fin_ps", tag="ep", space="PSUM")
                        nc.tensor.matmul(
                            out=fin_ps, lhsT=obt, rhs=owt_ext[s], start=True, stop=True
                        )
                        fin = ep_sb.tile([128, EMB], F32, name="fin")
                        nc.vector.tensor_copy(out=fin, in_=fin_ps)
                        nc.sync.dma_start(
                            out=out_t.ap()[si, 128 * j : 128 * (j + 1), :], in_=fin
                        )

    nc.compile()
    return nc


def _prep_core_inputs(inputs, core, ns):
    """Slice + marshal inputs for one core."""
    lo = core * ns
    hi = lo + ns
    m = {}
    for s, feats_name, cls_name, pre in (
        ("v", "v_s", "v_class", "var"),
        ("c", "c_s", "c_class", "con"),
    ):
        cls = np.asarray(inputs[cls_name][lo:hi]).astype(np.int16)
        m[f"f{s}"] = np.ascontiguousarray(np.asarray(inputs[feats_name][lo:hi], dtype=np.float32))
        # token -> slot mapping: slot (p, j) of supertile st = token 512*st + 4*p + j
        # (matches the contiguous feats DMA "(p j) i -> p j i" layout)
        # gather list position g of supertile st = slot (g % 128, g // 128)
        n_sup = ns // 512
        tok = (
            512 * np.arange(n_sup)[:, None]
            + 4 * (np.arange(512)[None, :] % 128)
            + (np.arange(512)[None, :] // 128)
        ).reshape(-1)  # [ns] token id at each gather-list position
        cls_list = cls[tok]
        wrapped = cls_list.reshape(ns // 16, 16).T  # [16, ns/16]
        m[f"gi_{s}"] = np.ascontiguousarray(np.tile(wrapped, (8, 1)))  # [128, ns/16]
        # cls_sb[p, 4*st + j] = cls[512*st + 4*p + j]
        m[f"cls_{s}"] = np.ascontiguousarray(
            cls.reshape(n_sup, 128, 4).transpose(1, 0, 2).reshape(128, -1).astype(np.float32)
        )
        m[f"sem_{s}"] = np.asarray(inputs[f"{'v' if s == 'v' else 'c'}_sem"], dtype=np.float32)
        m[f"inw_{s}"] = np.asarray(inputs[f"in_w_{pre}"], dtype=np.float32)
        m[f"inb_{s}"] = np.asarray(inputs[f"in_b_{pre}"], dtype=np.float32)
        m[f"outw_{s}"] = np.asarray(inputs[f"out_w_{pre}"], dtype=np.float32)
        m[f"outb_{s}"] = np.asarray(inputs[f"out_b_{pre}"], dtype=np.float32)
    return m


_CACHED = {}


def run(inputs, trace=False, **kw):
    from concourse.bass_utils import run_bass_kernel_spmd

    ns = N_ITEMS // N_CORES
    if "nc" not in _CACHED:
        _CACHED["nc"] = build_kernel(ns)
    nc = _CACHED["nc"]
    in_maps = [_prep_core_inputs(inputs, core, ns) for core in range(N_CORES)]
    return run_bass_kernel_spmd(
        nc, in_maps, core_ids=list(range(N_CORES)), trace=trace, **kw
    )


def kernel(**inputs):
    res = run(inputs)
    out = res.results[0]["out"]
    return (
        np.asarray(out[0], dtype=np.float32),
        np.asarray(out[1], dtype=np.float32),
    )


if __name__ == "__main__":
    pass
